# revision 3
# baseline (speedup 1.0000x reference)
"""DLoRF low-rank linear kernel for Trainium2 (8 NeuronCores, SPMD).

Computes  out = x @ U @ diag(s * mask) @ V.T  for
  x [8, 2048, 4096] f32, U [4096, 512], V [4096, 512], s/mask [512].

Strategy: data-parallel over the batch dim (one batch element per core).
Host folds diag(s*mask) into U (U_s = U * s_masked) and pre-transposes
V (Vt = V.T), both tiny. Per core:

  phase 1: stream x in natural layout, transpose 128x128 tiles on the
           PE (identity matmul) to get x.T tiles (feature-major), then
           GEMM1: tT[k', tok] += U_s[feat, k'].T @ xT[feat, tok]
  phase 2: GEMM2: out[tok, O] += tT[k', tok].T @ Vt[k', O], streamed
           over O chunks, DMA out.

Matmuls run as float32r (TF32-like: fp32 bits, mantissa rounded to
~12 bits inside the PE) which streams at 1 cycle/row -- 4x faster than
exact fp32. Measured rel-l2 error per GEMM ~1.5e-4.
"""

import numpy as np

import concourse.bacc as bacc
import concourse.mybir as mybir
import concourse.tile as tile
from concourse.bass_utils import run_bass_kernel_spmd
from concourse.masks import make_identity

B, S, IN_F, OUT_F, KR = 8, 2048, 4096, 4096, 512
P = 128
N_CORES = 8
KT = IN_F // P  # 32 feature tiles (contraction of GEMM1)
MT = KR // P  # 4 rank tiles (contraction of GEMM2)
CW = 256  # token chunk width (moving free dim of GEMM1)
CH = S // CW  # 8 chunks
OW = 512  # out-feature chunk width (moving free dim of GEMM2)
OC = OUT_F // OW  # 8 chunks

F32 = mybir.dt.float32
F32R = mybir.dt.float32r


def build(dt_mm=F32R, f32r_transpose=True):
    nc = bacc.Bacc()
    # dtype of the transpose path (x natural tiles, transpose psum)
    dt_tr = dt_mm if f32r_transpose else F32
    x_d = nc.declare_dram_parameter("x", [S, IN_F], dt_tr, isOutput=False)
    us_d = nc.declare_dram_parameter("us", [IN_F, KR], dt_mm, isOutput=False)
    vt_d = nc.declare_dram_parameter("vt", [KR, OUT_F], dt_mm, isOutput=False)
    out_d = nc.declare_dram_parameter("out", [S, OUT_F], F32, isOutput=True)

    with tile.TileContext(nc) as tc:
        with (
            tc.tile_pool(name="const", bufs=1) as constp,
            tc.tile_pool(name="wpool", bufs=1) as wpool,
            tc.tile_pool(name="xnat", bufs=3) as xnat_p,
            tc.tile_pool(name="xt", bufs=1) as xt_p,
            tc.tile_pool(name="tt", bufs=1) as tt_p,
            tc.tile_pool(name="vtp", bufs=2) as vt_p,
            tc.tile_pool(name="ostage", bufs=4) as ostage_p,
            tc.tile_pool(name="tps", bufs=2, space="PSUM") as tps,
            tc.tile_pool(name="ps1", bufs=2, space="PSUM") as ps1,
            tc.tile_pool(name="ps2", bufs=2, space="PSUM") as ps2,
        ):
            ident = constp.tile([P, P], F32)
            make_identity(nc, ident)
            if dt_tr == F32R:
                ident_mm = constp.tile([P, P], F32R)
                nc.gpsimd.dma_start(ident_mm[:], ident[:])
            else:
                ident_mm = ident

            us_t = wpool.tile([P, KT, KR], dt_mm)
            nc.sync.dma_start(us_t[:], us_d.rearrange("(kt p) m -> p kt m", p=P))

            tt_full = tt_p.tile([P, MT, S], dt_mm)

            # ---- phase 1: transpose x + GEMM1 ----
            for c in range(CH):
                xt_tile = xt_p.tile([P, KT, CW], dt_mm, tag="xt")
                for ts in range(CW // P):
                    tok0 = c * CW + ts * P
                    for half in range(2):
                        xn = xnat_p.tile([P, IN_F // 2], dt_tr, tag="xn")
                        nc.sync.dma_start(
                            xn[:],
                            x_d[tok0 : tok0 + P, half * 2048 : (half + 1) * 2048],
                        )
                        for q in range(4):
                            tp = tps.tile([P, 512], dt_tr, tag="tp")
                            for j in range(4):
                                nc.tensor.transpose(
                                    tp[:, j * P : (j + 1) * P],
                                    xn[:, (q * 4 + j) * P : (q * 4 + j + 1) * P],
                                    ident_mm,
                                )
                            kt0 = half * 16 + q * 4
                            nc.vector.tensor_copy(
                                xt_tile[:, kt0 : kt0 + 4, ts * P : (ts + 1) * P],
                                tp.rearrange("p (j c) -> p j c", j=4),
                            )
                for m in range(MT):
                    p1 = ps1.tile([P, CW], F32, tag="p1")
                    for kt in range(KT):
                        nc.tensor.matmul(
                            p1[:],
                            us_t[:, kt, m * P : (m + 1) * P],
                            xt_tile[:, kt, :],
                            start=(kt == 0),
                            stop=(kt == KT - 1),
                        )
                    nc.scalar.copy(tt_full[:, m, c * CW : (c + 1) * CW], p1[:])

            # ---- phase 2: GEMM2 + store ----
            vt_r = vt_d.rearrange("(mt p) o -> p mt o", p=P)
            for oc in range(OC):
                vt_t = vt_p.tile([P, MT, OW], dt_mm, tag="vt")
                nc.sync.dma_start(vt_t[:], vt_r[:, :, oc * OW : (oc + 1) * OW])
                for tb in range(S // P):
                    p2 = ps2.tile([P, OW], F32, tag="p2")
                    for m in range(MT):
                        nc.tensor.matmul(
                            p2[:],
                            tt_full[:, m, tb * P : (tb + 1) * P],
                            vt_t[:, m, :],
                            start=(m == 0),
                            stop=(m == MT - 1),
                        )
                    ost = ostage_p.tile([P, OW], F32, tag="ost")
                    nc.scalar.copy(ost[:], p2[:])
                    nc.sync.dma_start(
                        out_d[tb * P : (tb + 1) * P, oc * OW : (oc + 1) * OW],
                        ost[:],
                    )
    nc.finalize()
    return nc


_NC_CACHE = {}


def _get_nc():
    key = "main"
    if key not in _NC_CACHE:
        _NC_CACHE[key] = build()
    return _NC_CACHE[key]


def kernel(x, U, V, s, mask, _trace=False, _trace_kwargs=None):
    s_masked = (s.astype(np.float32) * mask.astype(np.float32)).astype(np.float32)
    U_s = np.ascontiguousarray(U.astype(np.float32) * s_masked[None, :])
    Vt = np.ascontiguousarray(V.astype(np.float32).T)
    nc = _get_nc()
    in_maps = [
        {"x": np.ascontiguousarray(x[b]), "us": U_s, "vt": Vt} for b in range(B)
    ]
    res = run_bass_kernel_spmd(
        nc, in_maps, list(range(N_CORES)), trace=_trace, **(_trace_kwargs or {})
    )
    out = np.stack([res.results[b]["out"] for b in range(B)], axis=0)
    if _trace:
        return out, res
    return out
